# revision 11
# baseline (speedup 1.0000x reference)
"""Trainium2 Bass kernel for nn_BatchRankingMSE_Loss (N=8192, 8 cores).

Math (margin M=2, eps=1e-4):
  mse     = mean((p-l)^2)
  T[i,j]  = relu(M - (p_j-p_i)*sign(l_j-l_i))   -- symmetric, T_ii = M
  ranking = sum_{i<j} T = (sum_all T - N*M)/2
  grad[i] = sum_j 1{M-(p_j-p_i)s_ij > 0}*s_ij   (antisymmetric full row sums)
  loss    = mse + (||g_mse|| / (||grad||+eps)) * ranking

Sharding: row-block data parallel; each of 8 cores evaluates its full
[8192 global-j x 1024 own-row] block. Tiles are [128 j-partitions x 1024
own-rows(free)]; per j-tile jt:
  s' = Sign(l_j - l_i)            ACT (f32 labels: exact tie handling)
  d  = p_i - p_j                  DVE tensor_scalar add   (bf16 4x)
  e  = d * s'                     DVE tensor_tensor mult  (bf16 2x)
  t  = max(e + M, 0) = relu(z)    DVE ts dual-op 4x / ACT Relu (alternating)
  u  = 1{e > -M} = relu'(z)       DVE ts is_gt 4x
  h  = u * s'                     DVE tensor_tensor mult  (bf16 2x)
  sum_j t, sum_j h                TensorE ones-matmul, PSUM-accumulated
                                  across all 64 j-tiles (partition reduce)
The host only folds [1 x 1024] per-core partial rows into the final scalar.
"""

import numpy as np
import ml_dtypes
from contextlib import ExitStack

MARGIN = 2.0
EPS = 1e-4
N = 8192
NCORES = 8
RPC = N // NCORES        # rows per core = 1024

_CACHE = {}
LAST_RESULTS = None      # test.py introspects timing from here


def build_nc(n=N, rpc=RPC):
    import concourse.bass as bass
    import concourse.mybir as mybir
    from concourse import bacc, tile

    dt = mybir.dt
    Af = mybir.ActivationFunctionType
    Op = mybir.AluOpType
    njt = n // 128           # j-tiles
    rt = rpc // 128          # row-tiles for the mse input layout

    nc = bacc.Bacc(None)
    pib_src = nc.dram_tensor("pib", [rpc], dt.bfloat16, kind="ExternalInput")
    lib_src = nc.dram_tensor("lib", [rpc], dt.float32, kind="ExternalInput")
    pneg_in = nc.dram_tensor("pneg", [128, njt], dt.float32, kind="ExternalInput")
    lj_in = nc.dram_tensor("lj", [128, njt], dt.float32, kind="ExternalInput")
    prow = nc.dram_tensor("prow", [128, rt], dt.float32, kind="ExternalInput")
    lrow = nc.dram_tensor("lrow", [128, rt], dt.float32, kind="ExternalInput")
    tsum_out = nc.dram_tensor("tsum", [1, rpc], dt.float32, kind="ExternalOutput")
    gsum_out = nc.dram_tensor("gsum", [1, rpc], dt.float32, kind="ExternalOutput")
    mse_out = nc.dram_tensor("msesq", [128, 1], dt.float32, kind="ExternalOutput")

    slab = min(512, rpc)     # PSUM-bank-sized column slabs
    nhalf = rpc // slab

    with tile.TileContext(nc) as tc:
        with (
            tc.tile_pool(name="persist", bufs=1) as pp,
            tc.tile_pool(name="work", bufs=3) as wp,
            tc.tile_pool(name="psum", bufs=1, space="PSUM") as qp,
        ):
            pib = pp.tile([128, rpc], dt.bfloat16)
            lib = pp.tile([128, rpc], dt.float32)
            pneg = pp.tile([128, njt], dt.float32)
            lj = pp.tile([128, njt], dt.float32)
            ones = pp.tile([128, 1], dt.bfloat16)
            pr = pp.tile([128, rt], dt.float32)
            lr = pp.tile([128, rt], dt.float32)
            dmse = pp.tile([128, rt], dt.float32)
            sqms = pp.tile([128, rt], dt.float32)
            msea = pp.tile([128, 1], dt.float32)
            marg = pp.tile([128, 1], dt.float32)
            tsb = pp.tile([1, rpc], dt.float32)
            gsb = pp.tile([1, rpc], dt.float32)

            tps = [qp.tile([1, slab], dt.float32, tag=f"tps{k}", name=f"tps{k}")
                   for k in range(nhalf)]
            gps = [qp.tile([1, slab], dt.float32, tag=f"gps{k}", name=f"gps{k}")
                   for k in range(nhalf)]

            nc.vector.memset(ones[:], 1.0)
            nc.vector.memset(marg[:], MARGIN)
            # broadcasts of this core's row block (stride-0 partition dim)
            half = rpc // 2
            for c in range(2):
                cs = slice(c * half, (c + 1) * half)
                nc.sync.dma_start(pib[:, cs], pib_src[cs].partition_broadcast(128))
                nc.sync.dma_start(lib[:, cs], lib_src[cs].partition_broadcast(128))
            nc.sync.dma_start(pneg[:], pneg_in[:])
            nc.sync.dma_start(lj[:], lj_in[:])
            nc.sync.dma_start(pr[:], prow[:])
            nc.sync.dma_start(lr[:], lrow[:])

            # mse partials: sum_free (p-l)^2 per partition
            nc.vector.scalar_tensor_tensor(
                dmse[:], pr[:], 0.0, lr[:], op0=Op.add, op1=Op.subtract)
            nc.vector.scalar_tensor_tensor(
                sqms[:], dmse[:], 1.0, dmse[:], op0=Op.mult, op1=Op.mult,
                accum_out=msea[:])
            nc.sync.dma_start(mse_out[:], msea[:])

            for jt in range(njt):
                s_t = wp.tile([128, rpc], dt.bfloat16, tag="s")
                d_t = wp.tile([128, rpc], dt.bfloat16, tag="d")
                e_t = wp.tile([128, rpc], dt.bfloat16, tag="e")
                t_t = wp.tile([128, rpc], dt.bfloat16, tag="t")
                u_t = wp.tile([128, rpc], dt.bfloat16, tag="u")
                h_t = wp.tile([128, rpc], dt.bfloat16, tag="h")
                # s' = sign(l_j - l_i)
                nc.scalar.activation(
                    s_t[:], lib[:], Af.Sign, bias=lj[:, jt:jt + 1], scale=-1.0)
                # d = p_i - p_j
                nc.vector.tensor_scalar(
                    d_t[:], pib[:], pneg[:, jt:jt + 1], None, op0=Op.add)
                # e = d * s'   (z = e + M)
                nc.vector.tensor_tensor(e_t[:], d_t[:], s_t[:], op=Op.mult)
                # t = relu(z): alternate engines to balance load
                if jt % 2 == 0:
                    nc.scalar.activation(
                        t_t[:], e_t[:], Af.Relu, bias=marg[:], scale=1.0)
                else:
                    nc.vector.tensor_scalar(
                        t_t[:], e_t[:], MARGIN, 0.0, op0=Op.add, op1=Op.max)
                # u = 1{z > 0}   (on GPSIMD: 1-input op at ~line rate,
                # frees DVE which is the bottleneck engine)
                nc.gpsimd.tensor_scalar(
                    u_t[:], e_t[:], -MARGIN, None, op0=Op.is_gt)
                # h = u * s'
                nc.vector.tensor_tensor(h_t[:], u_t[:], s_t[:], op=Op.mult)
                # partition-reduce into PSUM accumulators
                st = (jt == 0)
                sp = (jt == njt - 1)
                for k in range(nhalf):
                    cs = slice(k * slab, (k + 1) * slab)
                    nc.tensor.matmul(tps[k][:], ones[:], t_t[:, cs],
                                     start=st, stop=sp)
                    nc.tensor.matmul(gps[k][:], ones[:], h_t[:, cs],
                                     start=st, stop=sp)

            for k in range(nhalf):
                cs = slice(k * slab, (k + 1) * slab)
                nc.vector.tensor_copy(tsb[:, cs], tps[k][:])
                nc.vector.tensor_copy(gsb[:, cs], gps[k][:])
            nc.sync.dma_start(tsum_out[:], tsb[:])
            nc.sync.dma_start(gsum_out[:], gsb[:])
    if not nc.is_finalized():
        nc.finalize()
    return nc


def make_in_maps(preds, labels, ncores=NCORES, rpc=RPC):
    preds = np.asarray(preds, dtype=np.float32)
    labels = np.asarray(labels, dtype=np.float32)
    n = preds.shape[0]
    njt = n // 128
    rt = rpc // 128
    pneg = np.ascontiguousarray((-preds).reshape(njt, 128).T)
    lj = np.ascontiguousarray(labels.reshape(njt, 128).T)
    in_maps = []
    for c in range(ncores):
        rows = slice(c * rpc, (c + 1) * rpc)
        rp = preds[rows].reshape(rt, 128).T
        rl = labels[rows].reshape(rt, 128).T
        in_maps.append({
            "pib": preds[rows].astype(ml_dtypes.bfloat16),
            "lib": labels[rows],
            "pneg": pneg,
            "lj": lj,
            "prow": np.ascontiguousarray(rp),
            "lrow": np.ascontiguousarray(rl),
        })
    return in_maps


def combine(results, n=N):
    """Fold per-core partial sums into the scalar loss (host gather step)."""
    s_total = 0.0
    g2sq = 0.0
    msesum = 0.0
    for res in results:
        s_total += float(res["tsum"].astype(np.float64).sum())
        g = res["gsum"].astype(np.float64)
        g2sq += float((g * g).sum())
        msesum += float(res["msesq"].astype(np.float64).sum())
    ranking = (s_total - n * MARGIN) / 2.0
    g2 = np.sqrt(g2sq)
    mse = msesum / n
    g1 = 2.0 * np.sqrt(msesum) / n
    return np.float32(mse + (g1 / (g2 + EPS)) * ranking)


def kernel(preds, labels):
    global LAST_RESULTS
    from concourse.bass_utils import run_bass_kernel_spmd

    if "nc" not in _CACHE:
        _CACHE["nc"] = build_nc()
    in_maps = make_in_maps(preds, labels)
    res = run_bass_kernel_spmd(_CACHE["nc"], in_maps, list(range(NCORES)))
    LAST_RESULTS = res
    return combine(res.results)


# revision 12
# speedup vs baseline: 7.0542x; 7.0542x over previous
"""Trainium2 Bass kernel for nn_BatchRankingMSE_Loss (N=8192, 8 cores).

Math (margin M=2, eps=1e-4):
  mse     = mean((p-l)^2)
  T[i,j]  = relu(M - (p_j-p_i)*sign(l_j-l_i))   -- symmetric, T_ii = M
  ranking = sum_{i<j} T = (sum_all T - N*M)/2
  grad[i] = sum_j 1{M-(p_j-p_i)s_ij > 0}*s_ij   (antisymmetric full row sums)
  loss    = mse + (||g_mse|| / (||grad||+eps)) * ranking

Sharding: row-block data parallel; each of 8 cores evaluates its full
[8192 global-j x 1024 own-row] block. Tiles are [128 j-partitions x 1024
own-rows(free)]; per j-tile jt:
  s' = Sign(l_j - l_i)            ACT (f32 labels: exact tie handling)
  d  = p_i - p_j                  DVE tensor_scalar add   (bf16 4x)
  e  = d * s'                     DVE tensor_tensor mult  (bf16 2x)
  t  = max(e + M, 0) = relu(z)    DVE ts dual-op 4x / ACT Relu (alternating)
  u  = 1{e > -M} = relu'(z)       DVE ts is_gt 4x
  h  = u * s'                     DVE tensor_tensor mult  (bf16 2x)
  sum_j t, sum_j h                TensorE ones-matmul, PSUM-accumulated
                                  across all 64 j-tiles (partition reduce)
The host only folds [1 x 1024] per-core partial rows into the final scalar.
"""

import numpy as np
import ml_dtypes
from contextlib import ExitStack

MARGIN = 2.0
EPS = 1e-4
N = 8192
NCORES = 8
RPC = N // NCORES        # rows per core = 1024

_CACHE = {}
LAST_RESULTS = None      # test.py introspects timing from here


def build_nc(n=N, rpc=RPC):
    import concourse.bass as bass
    import concourse.mybir as mybir
    from concourse import bacc, tile

    dt = mybir.dt
    Af = mybir.ActivationFunctionType
    Op = mybir.AluOpType
    njt = n // 128           # j-tiles
    rt = rpc // 128          # row-tiles for the mse input layout

    nc = bacc.Bacc(None)
    pib_src = nc.dram_tensor("pib", [rpc], dt.bfloat16, kind="ExternalInput")
    lib_src = nc.dram_tensor("lib", [rpc], dt.float32, kind="ExternalInput")
    pneg_in = nc.dram_tensor("pneg", [128, njt], dt.float32, kind="ExternalInput")
    lj_in = nc.dram_tensor("lj", [128, njt], dt.float32, kind="ExternalInput")
    prow = nc.dram_tensor("prow", [128, rt], dt.float32, kind="ExternalInput")
    lrow = nc.dram_tensor("lrow", [128, rt], dt.float32, kind="ExternalInput")
    tsum_out = nc.dram_tensor("tsum", [1, rpc], dt.float32, kind="ExternalOutput")
    gsum_out = nc.dram_tensor("gsum", [1, rpc], dt.float32, kind="ExternalOutput")
    mse_out = nc.dram_tensor("msesq", [128, 1], dt.float32, kind="ExternalOutput")

    slab = min(512, rpc)     # PSUM-bank-sized column slabs
    nhalf = rpc // slab

    with tile.TileContext(nc) as tc:
        with (
            tc.tile_pool(name="persist", bufs=1) as pp,
            tc.tile_pool(name="work", bufs=3) as wp,
            tc.tile_pool(name="psum", bufs=1, space="PSUM") as qp,
        ):
            pib = pp.tile([128, rpc], dt.bfloat16)
            lib = pp.tile([128, rpc], dt.float32)
            pneg = pp.tile([128, njt], dt.float32)
            lj = pp.tile([128, njt], dt.float32)
            ones = pp.tile([128, 1], dt.bfloat16)
            pr = pp.tile([128, rt], dt.float32)
            lr = pp.tile([128, rt], dt.float32)
            dmse = pp.tile([128, rt], dt.float32)
            sqms = pp.tile([128, rt], dt.float32)
            msea = pp.tile([128, 1], dt.float32)
            marg = pp.tile([128, 1], dt.float32)
            tsb = pp.tile([1, rpc], dt.float32)
            gsb = pp.tile([1, rpc], dt.float32)

            tps = [qp.tile([1, slab], dt.float32, tag=f"tps{k}", name=f"tps{k}")
                   for k in range(nhalf)]
            gps = [qp.tile([1, slab], dt.float32, tag=f"gps{k}", name=f"gps{k}")
                   for k in range(nhalf)]

            nc.vector.memset(ones[:], 1.0)
            nc.vector.memset(marg[:], MARGIN)
            # broadcasts of this core's row block (stride-0 partition dim)
            half = rpc // 2
            for c in range(2):
                cs = slice(c * half, (c + 1) * half)
                nc.sync.dma_start(pib[:, cs], pib_src[cs].partition_broadcast(128))
                nc.sync.dma_start(lib[:, cs], lib_src[cs].partition_broadcast(128))
            nc.sync.dma_start(pneg[:], pneg_in[:])
            nc.sync.dma_start(lj[:], lj_in[:])
            nc.sync.dma_start(pr[:], prow[:])
            nc.sync.dma_start(lr[:], lrow[:])

            # mse partials: sum_free (p-l)^2 per partition
            nc.vector.scalar_tensor_tensor(
                dmse[:], pr[:], 0.0, lr[:], op0=Op.add, op1=Op.subtract)
            nc.vector.scalar_tensor_tensor(
                sqms[:], dmse[:], 1.0, dmse[:], op0=Op.mult, op1=Op.mult,
                accum_out=msea[:])
            nc.sync.dma_start(mse_out[:], msea[:])

            for jt in range(njt):
                s_t = wp.tile([128, rpc], dt.bfloat16, tag="s")
                d_t = wp.tile([128, rpc], dt.bfloat16, tag="d")
                e_t = wp.tile([128, rpc], dt.bfloat16, tag="e")
                t_t = wp.tile([128, rpc], dt.bfloat16, tag="t")
                u_t = wp.tile([128, rpc], dt.bfloat16, tag="u")
                h_t = wp.tile([128, rpc], dt.bfloat16, tag="h")
                # s' = sign(l_j - l_i)
                nc.scalar.activation(
                    s_t[:], lib[:], Af.Sign, bias=lj[:, jt:jt + 1], scale=-1.0)
                # d = p_i - p_j
                nc.vector.tensor_scalar(
                    d_t[:], pib[:], pneg[:, jt:jt + 1], None, op0=Op.add)
                # e = d * s'   (z = e + M)
                nc.vector.tensor_tensor(e_t[:], d_t[:], s_t[:], op=Op.mult)
                # t = relu(z): alternate engines to balance load
                if jt % 2 == 0:
                    nc.scalar.activation(
                        t_t[:], e_t[:], Af.Relu, bias=marg[:], scale=1.0)
                else:
                    nc.vector.tensor_scalar(
                        t_t[:], e_t[:], MARGIN, 0.0, op0=Op.add, op1=Op.max)
                # u = 1{z > 0}
                nc.vector.tensor_scalar(
                    u_t[:], e_t[:], -MARGIN, None, op0=Op.is_gt)
                # h = u * s'
                nc.vector.tensor_tensor(h_t[:], u_t[:], s_t[:], op=Op.mult)
                # partition-reduce into PSUM accumulators
                st = (jt == 0)
                sp = (jt == njt - 1)
                for k in range(nhalf):
                    cs = slice(k * slab, (k + 1) * slab)
                    nc.tensor.matmul(tps[k][:], ones[:], t_t[:, cs],
                                     start=st, stop=sp)
                    nc.tensor.matmul(gps[k][:], ones[:], h_t[:, cs],
                                     start=st, stop=sp)

            for k in range(nhalf):
                cs = slice(k * slab, (k + 1) * slab)
                nc.vector.tensor_copy(tsb[:, cs], tps[k][:])
                nc.vector.tensor_copy(gsb[:, cs], gps[k][:])
            nc.sync.dma_start(tsum_out[:], tsb[:])
            nc.sync.dma_start(gsum_out[:], gsb[:])
    if not nc.is_finalized():
        nc.finalize()
    return nc


def make_in_maps(preds, labels, ncores=NCORES, rpc=RPC):
    preds = np.asarray(preds, dtype=np.float32)
    labels = np.asarray(labels, dtype=np.float32)
    n = preds.shape[0]
    njt = n // 128
    rt = rpc // 128
    pneg = np.ascontiguousarray((-preds).reshape(njt, 128).T)
    lj = np.ascontiguousarray(labels.reshape(njt, 128).T)
    in_maps = []
    for c in range(ncores):
        rows = slice(c * rpc, (c + 1) * rpc)
        rp = preds[rows].reshape(rt, 128).T
        rl = labels[rows].reshape(rt, 128).T
        in_maps.append({
            "pib": preds[rows].astype(ml_dtypes.bfloat16),
            "lib": labels[rows],
            "pneg": pneg,
            "lj": lj,
            "prow": np.ascontiguousarray(rp),
            "lrow": np.ascontiguousarray(rl),
        })
    return in_maps


def combine(results, n=N):
    """Fold per-core partial sums into the scalar loss (host gather step)."""
    s_total = 0.0
    g2sq = 0.0
    msesum = 0.0
    for res in results:
        s_total += float(res["tsum"].astype(np.float64).sum())
        g = res["gsum"].astype(np.float64)
        g2sq += float((g * g).sum())
        msesum += float(res["msesq"].astype(np.float64).sum())
    ranking = (s_total - n * MARGIN) / 2.0
    g2 = np.sqrt(g2sq)
    mse = msesum / n
    g1 = 2.0 * np.sqrt(msesum) / n
    return np.float32(mse + (g1 / (g2 + EPS)) * ranking)


def kernel(preds, labels):
    global LAST_RESULTS
    from concourse.bass_utils import run_bass_kernel_spmd

    if "nc" not in _CACHE:
        _CACHE["nc"] = build_nc()
    in_maps = make_in_maps(preds, labels)
    res = run_bass_kernel_spmd(_CACHE["nc"], in_maps, list(range(NCORES)))
    LAST_RESULTS = res
    return combine(res.results)
